# revision 33
# baseline (speedup 1.0000x reference)
"""BlockSparseDilatedAttention TRN2 kernel (v2).

Full inputs q,k,v: [1, 8192, 12, 64] fp32. Output: same shape.

Math: 16 blocks of 512 tokens; block pairs (r, c) with |r-c| <= 2 (74 pairs).
Per pair, dilated segment attention in 3 head-groups of 4 heads:
  g0: seg 128, dil 1 -> 4 units of 128 tokens per block
  g1: seg 256, dil 2 -> 2 units of 128 (odd positions)
  g2: seg 128, dil 4 -> 1 unit of 128 (pos 2 mod 4), block-diag mask of 4x32
Each unit: softmax(Q K^T / 8) V over its own 128 kv tokens; pair outputs are
scatter-added into the query row block.  The g2 mask is folded into 5 extra
contraction rows (exact cancellation for same-subsegment, -512 otherwise).

Sharding: 8 cores = 2 head-halves x 4 row-quarters. Identical SPMD program;
edge cores get zero-padded kv blocks (zero V => zero contribution).

Optimizations vs the original baseline (82.5us -> 74.6us measured):
  - few large need-ordered input DMAs (2KB+/partition descriptors) plus tiny
    cb0/qt starter tiles, so the first S matmul starts ~10us (was 16.4us)
  - u-major input layouts so each sweep's data is one contiguous DMA slice
  - delta-sum rebalanced: g0 and all h1 reduces as GpSimd add trees, g1/g2 h0
    reduces on DVE - keeps the DVE prompt so the PE never stalls on ps_o
  - one output DMA per (sweep, row) covering both heads (g2: per head to
    shorten the tail)
Measured (ntff): ScalarE exp stream 44.4us busy is the critical path; PE
(S+AV matmul/LDW stream), DVE (recip+mul from PSUM) and GpSimd (trees) all
run ~90% dense inside that window. Rejected experiments: packing 2 units'
S^T into one PSUM tile for wider exps crashes HW (>=2 start=True matmuls
per bank); bf16 tmp gives no GpSimd speedup; single persistent 4-bank AV
PSUM tile serializes on its WAR edge (85-101us).
"""

import sys

import numpy as np

for _p in ("/opt/trn_rl_repo",):
    if _p not in sys.path:
        sys.path.append(_p)

# ---------------------------------------------------------------- constants
B, S, H, D = 1, 8192, 12, 64
BLOCK = 512
NB = S // BLOCK            # 16
GL = [512, 256, 128]       # gathered tokens per block, per group
GNU = [4, 2, 1]            # 128-token units per block, per group
GC = [64, 64, 69]          # contraction rows (g2 has 5 mask-aug rows)
MASK_M = 512.0
NCORES = 8
ROWS_PER_CORE = 4          # row blocks per quarter
CB = 8                     # col blocks per core (4 rows window, padded)
SCALE = 0.125              # 1/sqrt(64)

S_DTYPE = "f16"
AV_DTYPE = "f16"

# sweeps: (g, unit tuple). g0 split in two so AV starts early.
SWEEPS = [(0, (0, 1)), (0, (2, 3)), (1, (0, 1)), (2, (0,))]


def _gather_pos():
    pos = [np.arange(512)]
    pos.append(np.concatenate([s + 1 + 2 * np.arange(128) for s in (0, 256)]))
    pos.append(np.concatenate([s + 2 + 4 * np.arange(32) for s in (0, 128, 256, 384)]))
    return pos


POS = _gather_pos()


def _rows_for_cb(cb):
    """Local row indices i in [0,3] attending col block cb (c = 4R-2+cb)."""
    return max(0, cb - 4), min(3, cb)


# ---------------------------------------------------------------- bass build
_BASS_CACHE = {}


def _build_bass():
    if "nc" in _BASS_CACHE:
        return _BASS_CACHE["nc"]

    import concourse.tile as tile
    from concourse import bacc, mybir

    exp_fn = mybir.ActivationFunctionType.Exp
    AXIS_X = mybir.AxisListType.X
    f32 = mybir.dt.float32
    bf16 = mybir.dt.bfloat16
    dt_map = {"f32": f32, "f32r": mybir.dt.float32r, "bf16": bf16,
              "f16": mybir.dt.float16}
    sdt = dt_map[S_DTYPE]
    avdt = dt_map[AV_DTYPE]
    nc = bacc.Bacc("TRN2", target_bir_lowering=False, debug=False,
                   enable_asserts=False)

    # ---- DRAM tensors (u-major layouts so per-sweep slices are contiguous)
    # g<2: kt[g]: [128p(2h x 64d), nu*CB*128]   (u, cb, tok)
    #      qt[g]: [128p, nu*4*128]              (u, i, tok)
    #      v[g]:  [128p(k tokens), nu*2*CB*66]  (u, h, cb, tok|1)
    # g2:  kt2: [69, 2*CB*128] (h, cb, tok); qt2: [69, 2*4*128] (h, i, tok)
    #      v2:  [128, 1*2*CB*66]
    # out[g]: [128p(q), 4*nu*2*64] f32          (i, u, h, d)
    kt_d, qt_d, v_d, out_d = [], [], [], []
    for g in range(3):
        nu, C = GNU[g], GC[g]
        if g < 2:
            kt_d.append(nc.dram_tensor(f"kt{g}", [128, nu * CB * 128], sdt,
                                       kind="ExternalInput").ap())
            qt_d.append(nc.dram_tensor(f"qt{g}", [128, nu * 4 * 128], sdt,
                                       kind="ExternalInput").ap())
        else:
            kt_d.append(nc.dram_tensor(f"kt{g}", [C, 2 * CB * 128], sdt,
                                       kind="ExternalInput").ap())
            qt_d.append(nc.dram_tensor(f"qt{g}", [C, 2 * 4 * 128], sdt,
                                       kind="ExternalInput").ap())
        v_d.append(nc.dram_tensor(f"v{g}", [128, nu * 2 * CB * 66], avdt,
                                  kind="ExternalInput").ap())
        out_d.append(nc.dram_tensor(f"out{g}", [128, 4 * nu * 2 * 64], f32,
                                    kind="ExternalOutput").ap())

    with tile.TileContext(nc) as tc:
        with tc.tile_pool(name="inp", bufs=1) as inp, \
             tc.tile_pool(name="at", bufs=20) as atp, \
             tc.tile_pool(name="small", bufs=8) as small, \
             tc.tile_pool(name="ps", bufs=4, space="PSUM") as psp:

            # ---- SBUF input tiles, chunked by sweep need
            kt_sb, qt_sb, v_sb = {}, {}, {}
            kt00a = inp.tile([128, 1 * 128], sdt, name="kt00a", tag="kt00a")
            kt00b = inp.tile([128, 2 * 128], sdt, name="kt00b", tag="kt00b")
            kt00c = inp.tile([128, 5 * 128], sdt, name="kt00c", tag="kt00c")
            kt_sb[(0, 0)] = (kt00a, kt00b, kt00c)
            for g in range(2):
                nu = GNU[g]
                for u in range(nu):
                    if (g, u) == (0, 0):
                        continue
                    kt_sb[(g, u)] = inp.tile([128, CB * 128], sdt,
                                             name=f"kt{g}_{u}",
                                             tag=f"kt{g}_{u}")
                for u in range(nu):
                    qt_sb[(g, u)] = inp.tile([128, 4 * 128], sdt,
                                             name=f"qt{g}_{u}",
                                             tag=f"qt{g}_{u}")
            kt_sb[(2, 0)] = inp.tile([GC[2], 2 * CB * 128], sdt,
                                     name="kt2_0", tag="kt2_0")
            qt_sb[(2, 0)] = inp.tile([GC[2], 2 * 4 * 128], sdt,
                                     name="qt2_0", tag="qt2_0")
            for g, units in SWEEPS:
                u0 = units[0]
                nw = len(units)
                v_sb[(g, u0)] = inp.tile([128, nw * 2 * CB * 66], avdt,
                                         name=f"v{g}_{u0}",
                                         tag=f"v{g}_{u0}")

            # ---- input DMAs, need-ordered (single sync HWDGE queue)
            def dma_in(sb_t, dram, col0, col1):
                nc.sync.dma_start(out=sb_t, in_=dram[:, col0:col1])

            # sweep 0 first: cb0/i0 starters, then the rest in need order
            dma_in(kt00a, kt_d[0], 0, 1 * 128)
            dma_in(qt_sb[(0, 0)], qt_d[0], 0, 4 * 128)
            dma_in(kt00b, kt_d[0], 1 * 128, 3 * 128)
            dma_in(kt00c, kt_d[0], 3 * 128, CB * 128)
            dma_in(qt_sb[(0, 1)], qt_d[0], 4 * 128, 2 * 4 * 128)
            dma_in(kt_sb[(0, 1)], kt_d[0], CB * 128, 2 * CB * 128)
            dma_in(v_sb[(0, 0)], v_d[0], 0, 2 * 2 * CB * 66)
            # sweep 1
            dma_in(kt_sb[(0, 2)], kt_d[0], 2 * CB * 128, 3 * CB * 128)
            dma_in(qt_sb[(0, 2)], qt_d[0], 2 * 4 * 128, 3 * 4 * 128)
            dma_in(qt_sb[(0, 3)], qt_d[0], 3 * 4 * 128, 4 * 4 * 128)
            dma_in(kt_sb[(0, 3)], kt_d[0], 3 * CB * 128, 4 * CB * 128)
            dma_in(v_sb[(0, 2)], v_d[0], 2 * 2 * CB * 66, 4 * 2 * CB * 66)
            # sweep 2 (g1)
            dma_in(kt_sb[(1, 0)], kt_d[1], 0, CB * 128)
            dma_in(qt_sb[(1, 0)], qt_d[1], 0, 4 * 128)
            dma_in(qt_sb[(1, 1)], qt_d[1], 4 * 128, 2 * 4 * 128)
            dma_in(kt_sb[(1, 1)], kt_d[1], CB * 128, 2 * CB * 128)
            dma_in(v_sb[(1, 0)], v_d[1], 0, 2 * 2 * CB * 66)
            # sweep 3 (g2)
            dma_in(kt_sb[(2, 0)], kt_d[2], 0, 2 * CB * 128)
            dma_in(qt_sb[(2, 0)], qt_d[2], 0, 2 * 4 * 128)
            dma_in(v_sb[(2, 0)], v_d[2], 0, 2 * CB * 66)

            # ---- output SBUF accumulators
            out_sb = []
            for g in range(3):
                out_sb.append(inp.tile([128, 4 * GNU[g] * 2 * 64], f32,
                                       name=f"out{g}", tag=f"out{g}"))

            # ---- rearranged views
            # kt chunk [(g,u)]: [128, cb, 128]; qt chunk: [128, u-in-chunk, i, 128]
            # v chunk: [128, u-in-chunk, h, cb, 66]
            def kt_view(g, u):
                if (g, u) == (0, 0):
                    b = kt00b.rearrange("p (c l) -> p c l", c=2)
                    c = kt00c.rearrange("p (c l) -> p c l", c=5)
                    return lambda cb: (kt00a if cb == 0 else
                                       b[:, cb - 1] if cb < 3 else
                                       c[:, cb - 3])
                if g < 2:
                    t = kt_sb[(g, u)].rearrange("p (c l) -> p c l", c=CB)
                    return lambda cb: t[:, cb]
                t = kt_sb[(2, 0)].rearrange("p (h c l) -> p h c l", h=2, c=CB)
                return t

            def qt_view(g, u):
                if g < 2:
                    return qt_sb[(g, u)].rearrange("p (i l) -> p i l", i=4)
                return qt_sb[(2, 0)].rearrange("p (h i l) -> p h i l", h=2, i=4)

            def v_view(g, u0):
                t = v_sb[(g, u0)]
                return t.rearrange("p (u h c x) -> p u h c x", h=2, c=CB,
                                   u=t.shape[1] // (2 * CB * 66))

            out_v = [out_sb[g].rearrange("p (i u h x) -> p i u h x", i=4,
                                         u=GNU[g], h=2) for g in range(3)]
            out_dv = [out_d[g].rearrange("p (i u h x) -> p i u h x", i=4,
                                         u=GNU[g], h=2) for g in range(3)]

            at_tiles = {}
            for g, units in SWEEPS:
                nw = len(units)
                u0 = units[0]
                ktv = {u: kt_view(g, u) for u in units}
                qtv = {u: qt_view(g, u) for u in units}
                vv = v_view(g, u0)
                for cb in range(CB + 1):
                    i0, i1 = _rows_for_cb(min(cb, CB - 1))
                    nr = i1 - i0 + 1
                    # ---- scores S^T, both heads concurrent on PE row halves
                    if cb == CB:
                        pass
                    else:
                        for u in units:
                            sp = psp.tile([128, 2, 512], f32, tag="slot")
                            for h in range(2):
                                if g < 2:
                                    lhsT = ktv[u](cb)[64 * h:64 * h + 64, :]
                                    rhs = qtv[u][64 * h:64 * h + 64,
                                                 i0:i1 + 1, :]
                                else:
                                    lhsT = ktv[u][0:GC[2], h, cb, :]
                                    rhs = qtv[u][0:GC[2], h, i0:i1 + 1, :]
                                nc.tensor.matmul(sp[:, h, 0:nr * 128], lhsT,
                                                 rhs, start=True, stop=True)
                            at = atp.tile([128, 2, 512], avdt, tag="at")
                            nc.scalar.activation(at[:, :, 0:nr * 128],
                                                 sp[:, :, 0:nr * 128],
                                                 exp_fn, scale=SCALE)
                            at_tiles[(g, cb, u)] = at[:, :, 0:nr * 128]

                    # ---- AV + norm, delayed one cb so every at tile is
                    # at least one exp old (no PE wait on the fresh exp)
                    if cb < 5:
                        continue
                    i = cb - 5
                    for h in range(2):
                        po = psp.tile([128, 2, 512], f32, tag="slot")
                        for d in range(5):
                            ccb = i + d
                            ci0, _ = _rows_for_cb(ccb)
                            for us, u in enumerate(units):
                                a_t = at_tiles[(g, ccb, u)]
                                lhsT = a_t[:, h,
                                           (i - ci0) * 128:(i - ci0 + 1) * 128]
                                rhs = vv[:, u - u0, h, ccb, 0:65]
                                nc.tensor.matmul(po[:, us, d * 66:d * 66 + 65],
                                                 lhsT, rhs,
                                                 start=True, stop=True)
                        pv = po[:, :, 0:330].rearrange("p u (c x) -> p u c x",
                                                       x=66)
                        rc = small.tile([128, 2, 8], f32, tag="rc")
                        nc.vector.reciprocal_approx_fast(
                            out=rc[:, 0:nw, 0:5], in_=pv[:, 0:nw, :, 64])
                        tmp = small.tile([128, 2, 5, 64], f32, tag="tmp")
                        nc.vector.tensor_mul(
                            tmp[:, 0:nw], pv[:, 0:nw, :, 0:64],
                            rc[:, 0:nw, 0:5].broadcast_to([128, nw, 5, 64]))
                        dst = out_v[g][:, i, u0:u0 + nw, h, :]
                        if g == 0 or h == 1:
                            # delta-sum as an add tree on GpSimd
                            prs = tmp[:, 0:nw, 0:4, :].rearrange(
                                "p u (a b) d -> p u a b d", b=2)
                            sc = small.tile([128, 2, 2, 64], f32, tag="sc")
                            nc.gpsimd.tensor_add(sc[:, 0:nw],
                                                 prs[:, :, :, 0, :],
                                                 prs[:, :, :, 1, :])
                            sc2 = small.tile([128, 2, 64], f32, tag="sc2")
                            nc.gpsimd.tensor_add(sc2[:, 0:nw],
                                                 sc[:, 0:nw, 0, :],
                                                 sc[:, 0:nw, 1, :])
                            nc.gpsimd.tensor_add(dst, sc2[:, 0:nw],
                                                 tmp[:, 0:nw, 4, :])
                        else:
                            nc.vector.reduce_sum(
                                dst,
                                tmp[:, 0:nw].rearrange("p u c d -> p u d c"),
                                axis=AXIS_X)

                    # ship row i (both heads, this sweep's units)
                    if g == 2:
                        for hh in range(2):
                            nc.sync.dma_start(
                                out=out_dv[g][:, i, u0:u0 + nw, hh, :],
                                in_=out_v[g][:, i, u0:u0 + nw, hh, :])
                    else:
                        nc.sync.dma_start(out=out_dv[g][:, i, u0:u0 + nw, :, :],
                                          in_=out_v[g][:, i, u0:u0 + nw, :, :])

    nc.compile()
    _BASS_CACHE["nc"] = nc
    return nc


# ---------------------------------------------------------------- host pack
def _np_dtype(name):
    if name == "bf16":
        import ml_dtypes
        return ml_dtypes.bfloat16
    if name == "f16":
        return np.float16
    return np.float32


def _pack_inputs(q, k, v):
    """q,k,v: [1, 8192, 12, 64] fp32 -> list of 8 per-core input dicts."""
    q = np.asarray(q, dtype=np.float32)
    k = np.asarray(k, dtype=np.float32)
    v = np.asarray(v, dtype=np.float32)
    s_np = _np_dtype(S_DTYPE)
    av_np = _np_dtype(AV_DTYPE)
    qb = q.reshape(NB, BLOCK, H, D)
    kb = k.reshape(NB, BLOCK, H, D)
    vb = v.reshape(NB, BLOCK, H, D)

    # mask-augmentation constants for g2 (gathered order: 4 subsegs of 32)
    sub = np.repeat(np.arange(4), 32)                      # [128]
    U = (sub[None, :] == np.arange(4)[:, None]).astype(np.float32)  # [4,128]

    in_maps = []
    for core in range(NCORES):
        hh, R = core // 4, core % 4
        m = {}
        for g in range(3):
            nu, C = GNU[g], GC[g]
            pos = POS[g]
            L = 128  # gathered tokens per unit
            if g < 2:
                # kt: [128, nu, CB, 128]; qt: [128, nu, 4, 128]
                kt = np.zeros((128, nu, CB, L), np.float32)
                qt = np.zeros((128, nu, 4, L), np.float32)
            else:
                kt = np.zeros((C, 2, CB, L), np.float32)
                qt = np.zeros((C, 2, 4, L), np.float32)
            va = np.zeros((128, nu, 2, CB, 66), np.float32)
            for h in range(2):
                head = 4 * g + 2 * hh + h
                for i in range(ROWS_PER_CORE):
                    r = 4 * R + i
                    tok = qb[r, pos, head, :]              # [nu*128, 64]
                    for u in range(nu):
                        tu = tok[u * L:(u + 1) * L]        # [128, 64]
                        if g < 2:
                            qt[64 * h:64 * h + 64, u, i, :] = tu.T
                        else:
                            qt[0:64, h, i, :] = tu.T
                            qt[64, h, i, :] = -MASK_M
                            qt[65:69, h, i, :] = MASK_M * U
                for cb in range(CB):
                    c = 4 * R - 2 + cb
                    if 0 <= c < NB:
                        tok = kb[c, pos, head, :]
                        vt = vb[c, pos, head, :]
                    else:
                        tok = np.zeros((nu * L, D), np.float32)
                        vt = np.zeros((nu * L, D), np.float32)
                    for u in range(nu):
                        tu = tok[u * L:(u + 1) * L]
                        if g < 2:
                            kt[64 * h:64 * h + 64, u, cb, :] = tu.T
                        else:
                            kt[0:64, h, cb, :] = tu.T
                            kt[64, h, cb, :] = 1.0
                            kt[65:69, h, cb, :] = U
                        va[:, u, h, cb, 0:64] = vt[u * L:(u + 1) * L, :]
                        va[:, u, h, cb, 64] = 1.0
            m[f"kt{g}"] = kt.reshape(kt.shape[0], -1).astype(s_np)
            m[f"qt{g}"] = qt.reshape(qt.shape[0], -1).astype(s_np)
            m[f"v{g}"] = va.reshape(128, -1).astype(av_np)
        in_maps.append(m)
    return in_maps


def _unpack(results):
    out = np.zeros((B, S, H, D), np.float32)
    for core in range(NCORES):
        hh, R = core // 4, core % 4
        res = results[core]
        for g in range(3):
            nu = GNU[g]
            pos = POS[g]
            og = res[f"out{g}"].reshape(128, 4, nu, 2, 64)
            for h in range(2):
                head = 4 * g + 2 * hh + h
                for i in range(ROWS_PER_CORE):
                    r = 4 * R + i
                    for u in range(nu):
                        out[0, r * 512 + pos[u * 128:(u + 1) * 128], head, :] = \
                            og[:, i, u, h, :]
    return out


# ---------------------------------------------------------------- entry
def _run(q, k, v, trace=False):
    from concourse.bass_utils import run_bass_kernel_spmd
    nc = _build_bass()
    in_maps = _pack_inputs(q, k, v)
    res = run_bass_kernel_spmd(nc, in_maps, core_ids=list(range(NCORES)),
                               trace=trace)
    return _unpack(res.results), res


def kernel(q, k, v):
    out, _ = _run(q, k, v, trace=False)
    return out
